# revision 14
# baseline (speedup 1.0000x reference)
"""Trainium2 Bass kernel for nn_MessagePassingLayer (gnn_message_passing).

Strategy: shard the (batch, node_i) pairs across 8 NeuronCores (96 pairs/core).
Each core handles the full edge-MLP for its row-block of edges plus the node
update for its nodes. No cross-core communication is needed (the j-sum in the
aggregate is local to a row block); host assembles the shards.

Layouts (per (b,i) pair, j = neighbor index):
  - edges arrive both normal [j, e] (for the adjacency aggregate) and
    host-pre-transposed [e, j] (GEMM1 stationary + residual).
  - GEMM1 (edge MLP in): he[j-tile, h] = edgesT_aug.T @ We_aug  accumulated with
    an identity-matmul inject of njs (nodes@Wj, precomputed on device) and the
    ones-row trick for (ni + eb1).
  - LayerNorm stats via bn_stats/bn_aggr (per-partition = per edge row), then a
    single ScalarE activation applies (x-mu)*rstd with fused ReLU, writing bf16.
  - relu_he is transposed [j,h]->[h,j] by the DMA xbar (2-byte transpose engine)
    so GEMM2 can contract over h with ew2 stationary: out[e, j].
  - Residual + PSUM evict fused in one VectorE tensor_add; store is contiguous
    into an [e, j]-layout DRAM output that the host untransposes.
"""

import os
import sys
from contextlib import ExitStack

import numpy as np

os.environ.setdefault("MYCRO_LOCAL_CACHE", "1")

for _p in ("/opt/trn_rl_repo", "/root/.axon_site/_ro/trn_rl_repo"):
    if os.path.isdir(_p) and _p not in sys.path:
        sys.path.insert(0, _p)

import concourse.bass as bass
import concourse.bacc as bacc
import concourse.tile as tile
from concourse import mybir
from concourse.masks import make_identity

B, N, D, E, H = 2, 384, 768, 64, 512
LN_EPS = 1e-5
NCORES = 8
PAIRS = B * N            # 768 (b,i) pairs
PPC = PAIRS // NCORES    # 96 pairs per core
P = 128
JT = N // P              # 3 j-tiles per pair
HC = H // P              # 4 h-chunks
DC = D // P              # 6 d-chunks

F32 = mybir.dt.float32
F32R = mybir.dt.float32r
BF16 = mybir.dt.bfloat16
FP16 = mybir.dt.float16

# ---- tunables -------------------------------------------------------------
GEMM_DT = FP16    # dtype for the big GEMM1 / inject matmuls (full PE rate, half DMA)
RELU_DT = FP16    # relu_he storage + GEMM2 dtype (BF16 enables xbar transpose)
EDGES_BUFS = 4    # ring depth for edgesT tiles
WE_BUFS = 3       # ring depth for We_aug (row 64 rewritten per pair)
NORM_BUFS = 3     # ring depth for normal-layout edge tiles (agg)

_CACHE = {}


def _build(flags):
    """Build + compile the SPMD program. flags: (edge_affine, node_affine)."""
    edge_affine, node_affine = flags
    nc = bacc.Bacc("TRN2", target_bir_lowering=False, debug=False,
                   num_devices=NCORES)

    dt_np = mybir.dt.np

    def rd(ap):
        # view for vector-engine reads of GEMM_DT tiles
        return ap.bitcast(F32) if GEMM_DT == F32R else ap

    def din(name, shape, dt):
        return nc.dram_tensor(name, list(shape), dt, kind="ExternalInput").ap()

    def dout(name, shape, dt):
        return nc.dram_tensor(name, list(shape), dt, kind="ExternalOutput").ap()

    # ---- DRAM parameters (per-core shard shapes) ----
    edgesT = din("edgesT", (PPC, E, N), GEMM_DT)     # host-transposed edges shard
    edgesT32 = din("edgesT32", (PPC, E, N), F32)  # fp32 copy for exact residual
    edgesN = din("edgesN", (PPC, N, E), GEMM_DT)     # normal layout (for agg)
    adj = din("adj", (PPC, N), GEMM_DT)
    nodesTb = din("nodesTb", (D, N), GEMM_DT)        # nodes[b_core].T (for njs)
    nodesTsh = din("nodesTsh", (D, PPC), GEMM_DT)
    nodesTsh32 = din("nodesTsh32", (D, PPC), F32)    # this core's nodes, transposed
    we = din("we", (E, H), GEMM_DT)                  # ew1[:E]
    wi = din("wi", (D, H), GEMM_DT)                  # ew1[E:E+D]
    wj = din("wj", (D, H), GEMM_DT)                  # ew1[E+D:]
    ew2b = din("ew2b", (H, E), RELU_DT)
    eb1r = din("eb1r", (1, H), GEMM_DT)
    eb2r = din("eb2r", (1, E), GEMM_DT)
    nw1a = din("nw1a", (E, H), GEMM_DT)              # nw1[:E]
    nw1n = din("nw1n", (D, H), GEMM_DT)              # nw1[E:] * N (scale folded)
    nw2b = din("nw2b", (H, D), RELU_DT)
    nb1r = din("nb1r", (1, H), GEMM_DT)
    nb2r = din("nb2r", (1, D), GEMM_DT)
    # LN affine params (only used when not trivially ones/zeros)
    eg_r = din("eg_r", (1, H), F32)
    ebt_r = din("ebt_r", (1, H), F32)
    ng_r = din("ng_r", (1, H), F32)
    nbt_r = din("nbt_r", (1, H), F32)

    edgesT_out = dout("edgesT_out", (PPC, E, N), F32)
    nodesT_out = dout("nodesT_out", (D, PPC), F32)

    with tile.TileContext(nc) as tc, ExitStack() as ctx:
        consts = ctx.enter_context(tc.tile_pool(name="consts", bufs=1))
        ring = ctx.enter_context(tc.tile_pool(name="ring", bufs=1))
        adj_pool = ctx.enter_context(tc.tile_pool(name="adjp", bufs=3))
        spool = ctx.enter_context(tc.tile_pool(name="stats", bufs=12))
        relu_pool = ctx.enter_context(tc.tile_pool(name="relu", bufs=4))
        rht_pool = ctx.enter_context(tc.tile_pool(name="rht", bufs=2))
        out_pool = ctx.enter_context(tc.tile_pool(name="outs", bufs=3))
        he_psum = ctx.enter_context(tc.tile_pool(name="hep", bufs=2, space="PSUM"))
        oe_psum = ctx.enter_context(tc.tile_pool(name="oep", bufs=1, space="PSUM"))
        agg_psum = ctx.enter_context(tc.tile_pool(name="aggp", bufs=1, space="PSUM"))
        np_psum = ctx.enter_context(tc.tile_pool(name="npp", bufs=1, space="PSUM"))
        nd_psum = ctx.enter_context(tc.tile_pool(name="ndp", bufs=2, space="PSUM"))

        # ---- constants / static SBUF ----
        ident = consts.tile([P, P], GEMM_DT)
        make_identity(nc, ident)
        ones_n = consts.tile([1, N], GEMM_DT)
        nc.vector.memset(ones_n, 1.0)
        ones_p = consts.tile([1, PPC], GEMM_DT)
        nc.vector.memset(ones_p, 1.0)
        eps_t = consts.tile([P, 1], F32)
        nc.vector.memset(eps_t, LN_EPS)

        eb1_sb = consts.tile([1, H], GEMM_DT)
        nc.sync.dma_start(out=eb1_sb, in_=eb1r)
        eb2_sb = consts.tile([1, E], GEMM_DT)
        nc.sync.dma_start(out=eb2_sb, in_=eb2r)
        nb1_sb = consts.tile([1, H], GEMM_DT)
        nc.sync.dma_start(out=nb1_sb, in_=nb1r)
        nb2_sb = consts.tile([1, D], GEMM_DT)
        nc.sync.dma_start(out=nb2_sb, in_=nb2r)

        wj_sb = consts.tile([P, DC, H], GEMM_DT)
        nc.sync.dma_start(out=wj_sb, in_=wj.rearrange("(c p) h -> p c h", p=P))
        wi_sb = consts.tile([P, DC, H], GEMM_DT)
        nc.sync.dma_start(out=wi_sb, in_=wi.rearrange("(c p) h -> p c h", p=P))
        ntb_sb = consts.tile([P, DC, N], GEMM_DT)
        nc.sync.dma_start(out=ntb_sb, in_=nodesTb.rearrange("(c p) n -> p c n", p=P))
        nst_sb = consts.tile([P, DC, PPC], GEMM_DT)
        nc.sync.dma_start(out=nst_sb, in_=nodesTsh.rearrange("(c p) n -> p c n", p=P))
        nst32_sb = consts.tile([P, DC, PPC], F32)
        nc.sync.dma_start(out=nst32_sb,
                          in_=nodesTsh32.rearrange("(c p) n -> p c n", p=P))
        ew2_sb = consts.tile([P, HC, E], RELU_DT)
        nc.sync.dma_start(out=ew2_sb, in_=ew2b.rearrange("(c p) e -> p c e", p=P))
        nw1a_sb = consts.tile([E, H], GEMM_DT)
        nc.sync.dma_start(out=nw1a_sb, in_=nw1a)
        nw1n_sb = consts.tile([P, DC, H], GEMM_DT)
        nc.sync.dma_start(out=nw1n_sb, in_=nw1n.rearrange("(c p) h -> p c h", p=P))
        nw2_sb = consts.tile([P, HC, D], RELU_DT)
        nc.sync.dma_start(out=nw2_sb, in_=nw2b.rearrange("(c p) d -> p c d", p=P))

        if edge_affine:
            eg_sb = consts.tile([P, H], F32)
            nc.sync.dma_start(out=eg_sb, in_=eg_r.to_broadcast((P, H)))
            ebt_sb = consts.tile([P, H], F32)
            nc.sync.dma_start(out=ebt_sb, in_=ebt_r.to_broadcast((P, H)))
        if node_affine:
            ng_sb = consts.tile([P, H], F32)
            nc.sync.dma_start(out=ng_sb, in_=ng_r.to_broadcast((P, H)))
            nbt_sb = consts.tile([P, H], F32)
            nc.sync.dma_start(out=nbt_sb, in_=nbt_r.to_broadcast((P, H)))

        # ---- device precompute: njs[b_core] and nis (+eb1) ----
        njs_sb = consts.tile([P, JT, H], GEMM_DT)   # nodes[b] @ Wj, [j, h]
        for jt in range(JT):
            ps = np_psum.tile([P, H], F32, tag="np_ps")
            for c in range(DC):
                nc.tensor.matmul(ps, lhsT=ntb_sb[:, c, bass.ts(jt, P)],
                                 rhs=wj_sb[:, c, :],
                                 start=(c == 0), stop=(c == DC - 1))
            nc.scalar.copy(out=njs_sb[:, jt, :], in_=ps)

        nis_sb = consts.tile([PPC, H], GEMM_DT)     # nodes_sh @ Wi + eb1
        ps = np_psum.tile([PPC, H], F32, tag="np_ps")
        for c in range(DC):
            nc.tensor.matmul(ps, lhsT=nst_sb[:, c, :], rhs=wi_sb[:, c, :],
                             start=(c == 0), stop=False)
        nc.tensor.matmul(ps, lhsT=ones_p, rhs=eb1_sb, start=False, stop=True)
        nc.scalar.copy(out=nis_sb, in_=ps)

        # ---- rings with preset ones-rows ----
        eT_slots = []
        for r in range(EDGES_BUFS):
            t = ring.tile([E + 1, N], GEMM_DT, tag=f"eT{r}")
            nc.vector.memset(t[E:E + 1, :], 1.0)   # ones row for the ni inject
            eT_slots.append(t)
        we_slots = []
        for r in range(WE_BUFS):
            t = ring.tile([E + 1, H], GEMM_DT, tag=f"we{r}")
            nc.sync.dma_start(out=t[0:E, :], in_=we)
            we_slots.append(t)
        eN_slots = [ring.tile([P, JT, E], GEMM_DT, tag=f"eN{r}", name=f"eN{r}")
                    for r in range(NORM_BUFS)]
        eT32_slots = [ring.tile([E, N], F32, tag=f"eT32_{r}", name=f"eT32_{r}")
                      for r in range(NORM_BUFS)]

        agg_ps = agg_psum.tile([E, PPC], F32)

        # ================= main edge loop =================
        for bi in range(PPC):
            eT = eT_slots[bi % EDGES_BUFS]
            nc.gpsimd.dma_start(out=eT[0:E, :], in_=edgesT[bi])
            eN = eN_slots[bi % NORM_BUFS]
            nc.gpsimd.dma_start(
                out=eN, in_=edgesN[bi].rearrange("(t p) e -> p t e", p=P))
            eT32 = eT32_slots[bi % NORM_BUFS]
            nc.gpsimd.dma_start(out=eT32, in_=edgesT32[bi])
            adj_t = adj_pool.tile([P, JT], GEMM_DT, tag="adj")
            nc.gpsimd.dma_start(
                out=adj_t, in_=adj[bi].rearrange("(t p) -> p t", p=P))
            weS = we_slots[bi % WE_BUFS]
            nc.sync.dma_start(out=weS[E:E + 1, :], in_=nis_sb[bi:bi + 1, :])

            # adjacency aggregate: agg[:, bi] += edges[j,:].T @ adj[j]
            for t in range(JT):
                nc.tensor.matmul(agg_ps[:, bi:bi + 1], lhsT=eN[:, t, :],
                                 rhs=adj_t[:, t:t + 1],
                                 start=(t == 0), stop=(t == JT - 1),
                                 skip_group_check=True)

            rht = [rht_pool.tile([P, N], RELU_DT, tag=f"rht{k}", name=f"rht{k}")
                   for k in range(HC)]
            for jt in range(JT):
                he = he_psum.tile([P, H], F32, tag="he")
                nc.tensor.matmul(he, lhsT=eT[:, bass.ts(jt, P)], rhs=weS,
                                 start=True, stop=False)
                nc.tensor.matmul(he, lhsT=ident, rhs=njs_sb[:, jt, :],
                                 start=False, stop=True)
                # LayerNorm stats
                st = spool.tile([P, 6], F32, tag="st")
                nc.vector.bn_stats(out=st, in_=he)
                mv = spool.tile([P, 2], F32, tag="mv")
                nc.vector.bn_aggr(out=mv, in_=st)
                sd = spool.tile([P, 1], F32, tag="sd")
                nc.scalar.activation(sd, mv[:, 1:2],
                                     mybir.ActivationFunctionType.Sqrt,
                                     bias=eps_t)
                rstd = spool.tile([P, 1], F32, tag="rstd")
                nc.vector.reciprocal(out=rstd, in_=sd)
                nb = spool.tile([P, 1], F32, tag="nb")
                nc.vector.tensor_scalar(out=nb, in0=mv[:, 0:1], scalar1=rstd,
                                        scalar2=-1.0,
                                        op0=mybir.AluOpType.mult,
                                        op1=mybir.AluOpType.mult)
                rh = relu_pool.tile([P, H], RELU_DT, tag="rh")
                if not edge_affine:
                    nc.scalar.activation(rh, he,
                                         mybir.ActivationFunctionType.Relu,
                                         bias=nb, scale=rstd)
                else:
                    t0 = relu_pool.tile([P, H], F32, tag="t0")
                    nc.scalar.activation(t0, he,
                                         mybir.ActivationFunctionType.Identity,
                                         bias=nb, scale=rstd)
                    nc.vector.tensor_tensor(out=t0, in0=t0, in1=eg_sb,
                                            op=mybir.AluOpType.mult)
                    nc.vector.tensor_tensor(out=t0, in0=t0, in1=ebt_sb,
                                            op=mybir.AluOpType.add)
                    nc.scalar.activation(rh, t0,
                                         mybir.ActivationFunctionType.Relu)
                # transpose [j,h] -> [h,j] via the DMA xbar (2-byte only)
                for k in range(HC):
                    nc.sync.dma_start_transpose(
                        out=rht[k][:, bass.ts(jt, P)],
                        in_=rh[:, bass.ts(k, P)])

            # GEMM2: out[e, j] = relu_he @ ew2 (+ eb2) + residual
            oe = oe_psum.tile([E, N], F32, tag="oe")
            for k in range(HC):
                nc.tensor.matmul(oe, lhsT=ew2_sb[:, k, :], rhs=rht[k],
                                 start=(k == 0), stop=False)
            nc.tensor.matmul(oe, lhsT=eb2_sb, rhs=ones_n,
                             start=False, stop=True)
            osb = out_pool.tile([E, N], F32, tag="osb")
            nc.vector.tensor_tensor(out=osb, in0=oe,
                                    in1=eT32,
                                    op=mybir.AluOpType.add)
            nc.gpsimd.dma_start(out=edgesT_out[bi], in_=osb)

        # ================= node update =================
        agg_sb = consts.tile([E, PPC], GEMM_DT)
        nc.vector.tensor_copy(out=agg_sb, in_=agg_ps)

        h1 = np_psum.tile([PPC, H], F32, tag="np_ps")
        nc.tensor.matmul(h1, lhsT=agg_sb, rhs=nw1a_sb, start=True, stop=False)
        for c in range(DC):
            nc.tensor.matmul(h1, lhsT=nst_sb[:, c, :], rhs=nw1n_sb[:, c, :],
                             start=False, stop=False)
        nc.tensor.matmul(h1, lhsT=ones_p, rhs=nb1_sb, start=False, stop=True)

        st = spool.tile([PPC, 6], F32, tag="nst")
        nc.vector.bn_stats(out=st, in_=h1)
        mv = spool.tile([PPC, 2], F32, tag="nmv")
        nc.vector.bn_aggr(out=mv, in_=st)
        sd = spool.tile([PPC, 1], F32, tag="nsd")
        nc.scalar.activation(sd, mv[:, 1:2], mybir.ActivationFunctionType.Sqrt,
                             bias=eps_t[0:PPC, :])
        rstd = spool.tile([PPC, 1], F32, tag="nrstd")
        nc.vector.reciprocal(out=rstd, in_=sd)
        nb = spool.tile([PPC, 1], F32, tag="nnb")
        nc.vector.tensor_scalar(out=nb, in0=mv[:, 0:1], scalar1=rstd,
                                scalar2=-1.0, op0=mybir.AluOpType.mult,
                                op1=mybir.AluOpType.mult)
        rh = relu_pool.tile([PPC, H], RELU_DT, tag="nrh")
        if not node_affine:
            nc.scalar.activation(rh, h1, mybir.ActivationFunctionType.Relu,
                                 bias=nb, scale=rstd)
        else:
            t0 = relu_pool.tile([PPC, H], F32, tag="nt0")
            nc.scalar.activation(t0, h1,
                                 mybir.ActivationFunctionType.Identity,
                                 bias=nb, scale=rstd)
            nc.vector.tensor_tensor(out=t0, in0=t0, in1=ng_sb[0:PPC, :],
                                    op=mybir.AluOpType.mult)
            nc.vector.tensor_tensor(out=t0, in0=t0, in1=nbt_sb[0:PPC, :],
                                    op=mybir.AluOpType.add)
            nc.scalar.activation(rh, t0, mybir.ActivationFunctionType.Relu)

        rhtn = [rht_pool.tile([P, PPC], RELU_DT, tag=f"nrht{k}",
                              name=f"nrht{k}") for k in range(HC)]
        for k in range(HC):
            nc.sync.dma_start_transpose(out=rhtn[k], in_=rh[:, bass.ts(k, P)])

        for c in range(DC):
            nd = nd_psum.tile([P, PPC], F32, tag="nd")
            for k in range(HC):
                nc.tensor.matmul(nd, lhsT=nw2_sb[:, k, bass.ts(c, P)],
                                 rhs=rhtn[k], start=(k == 0), stop=False)
            nc.tensor.matmul(nd, lhsT=nb2_sb[:, bass.ts(c, P)], rhs=ones_p,
                             start=False, stop=True)
            ndsb = out_pool.tile([P, PPC], F32, tag="ndsb")
            nc.vector.tensor_tensor(out=ndsb, in0=nd,
                                    in1=nst32_sb[:, c, :],
                                    op=mybir.AluOpType.add)
            nc.gpsimd.dma_start(out=nodesT_out[bass.ts(c, P), :], in_=ndsb)

    nc.compile()
    return nc


def get_nc(flags=(False, False)):
    if flags not in _CACHE:
        _CACHE[flags] = _build(flags)
    return _CACHE[flags]


def make_in_maps(nodes, edges, adjacency, nw1, nb1, ng, nbt, nw2, nb2,
                 ew1, eb1, eg, ebt, ew2, eb2):
    import ml_dtypes
    bf16 = ml_dtypes.bfloat16
    f32 = np.float32
    gnp = {BF16: bf16, FP16: np.float16, F32R: f32, F32: f32}[GEMM_DT]

    relu_np = {BF16: bf16, FP16: np.float16, F32R: f32}[RELU_DT]

    def cast_relu(x):
        return np.ascontiguousarray(x).astype(relu_np)

    edges_f = edges.reshape(PAIRS, N, E)
    adj_f = adjacency.reshape(PAIRS, N)
    nodes_f = nodes.reshape(PAIRS, D)
    we, wi, wj = ew1[:E], ew1[E:E + D], ew1[E + D:]
    shared = dict(
        we=np.ascontiguousarray(we, gnp),
        wi=np.ascontiguousarray(wi, gnp),
        wj=np.ascontiguousarray(wj, gnp),
        ew2b=cast_relu(ew2),
        eb1r=np.ascontiguousarray(eb1[None, :], gnp),
        eb2r=np.ascontiguousarray(eb2[None, :], gnp),
        nw1a=np.ascontiguousarray(nw1[:E], gnp),
        nw1n=np.ascontiguousarray(nw1[E:] * np.float32(N), gnp),
        nw2b=cast_relu(nw2),
        nb1r=np.ascontiguousarray(nb1[None, :], gnp),
        nb2r=np.ascontiguousarray(nb2[None, :], gnp),
        eg_r=np.ascontiguousarray(eg[None, :], f32),
        ebt_r=np.ascontiguousarray(ebt[None, :], f32),
        ng_r=np.ascontiguousarray(ng[None, :], f32),
        nbt_r=np.ascontiguousarray(nbt[None, :], f32),
    )
    in_maps = []
    for c in range(NCORES):
        sl = slice(c * PPC, (c + 1) * PPC)
        b_core = (c * PPC) // N
        m = dict(shared)
        eT_f32 = np.ascontiguousarray(edges_f[sl].transpose(0, 2, 1), f32)
        m["edgesT"] = eT_f32.astype(gnp)
        m["edgesT32"] = eT_f32
        m["edgesN"] = np.ascontiguousarray(edges_f[sl]).astype(gnp)
        m["adj"] = np.ascontiguousarray(adj_f[sl]).astype(gnp)
        m["nodesTb"] = np.ascontiguousarray(nodes[b_core].T).astype(gnp)
        nsh32 = np.ascontiguousarray(nodes_f[sl].T, f32)
        m["nodesTsh"] = nsh32.astype(gnp)
        m["nodesTsh32"] = nsh32
        in_maps.append(m)
    return in_maps


def run(inputs, trace=False):
    """Run on 8 cores; returns (nodes_updated, edges_updated, results_obj)."""
    from concourse.bass_utils import run_bass_kernel_spmd
    from concourse.bass_interp import get_hw_module

    edge_affine = not (np.all(inputs["eg"] == 1) and np.all(inputs["ebt"] == 0))
    node_affine = not (np.all(inputs["ng"] == 1) and np.all(inputs["nbt"] == 0))
    nc = get_nc((edge_affine, node_affine))
    in_maps = make_in_maps(**inputs)

    old_m = nc.m
    nc.m = get_hw_module(nc.m)
    try:
        res = run_bass_kernel_spmd(nc, in_maps, list(range(NCORES)),
                                   trace=trace)
    finally:
        nc.m = old_m

    edges_out = np.empty((PAIRS, N, E), np.float32)
    nodes_out = np.empty((PAIRS, D), np.float32)
    for c in range(NCORES):
        sl = slice(c * PPC, (c + 1) * PPC)
        edges_out[sl] = res.results[c]["edgesT_out"].transpose(0, 2, 1)
        nodes_out[sl] = res.results[c]["nodesT_out"].T
    return (nodes_out.reshape(B, N, D),
            edges_out.reshape(B, N, N, E),
            res)


def kernel(**inputs):
    inputs = {k: np.asarray(v) for k, v in inputs.items()}
    nodes_upd, edges_upd, _ = run(inputs, trace=False)
    return nodes_upd, edges_upd
